# revision 9
# baseline (speedup 1.0000x reference)
"""AudioSNN Trainium2 kernel (v6: packed fp16-split matmuls, batch-major L2).

Two-layer leaky-integrate-and-fire SNN (snntorch Leaky, reset-by-subtract),
T=500 recurrent steps over batch 4096, data-parallel over 8 NeuronCores
(512 batch elements per core).

Math (per step t, reference):
    cur1 = x_t @ W1.T + b1
    m1   = beta*m1 + cur1 - spk1[t-1]
    spk1 = H(m1 - 1)
    cur2 = spk1 @ W2.T + b2
    m2   = beta*m2 + cur2 - spk2[t-1]
    spk2 = H(m2 - 1)    -> output [T, B, 5]

Device formulation (per core, fp16 matmuls with hi/lo splits -> ~1e-7
error, f32 state):

  L1 (feature-major [128, 512], spikes as sgn = sign(z1) in {-1,+1}):
    p1 = one packed matmul (K=122):
         W1hi^T@xhi + bias_hi + W1hi^T@xlo + W1lo^T@xhi + bias_lo
    z1q  = beta*z1[t-1] + p1          (DVE stt)
    z1[t] = -0.5*sg1[t-1] + z1q       (DVE stt: reset-by-subtract)
    sg1[t] = Sign(z1[t]) fp16         (ACT)

  L2 (batch-major [128, 20]: partition = b%128, free = 5*(b//128)+o,
      spikes as 0/1):
    p2t = (ones2^T@c2rows)            (K=2 matmul: per-o constant c2)
        + per batch group g: sg1[:,g]^T @ w2hi + sg1[:,g]^T @ w2lo
          (spikes loaded as PE weights; out partitions = batch)
    q    = beta*Y[u-1] + p2t          (DVE stt)
    Y[u] = q - spk[u-1]               (GpSimd tensor_tensor, fp16 0/1 spk)
    spk[u] = (Y[u] > 0) fp16          (DVE tensor_single_scalar) -> output

  L2 runs LAG=2 steps behind L1 so its chain overlaps the L1 recurrence.
  Output is 0/1 fp16 in [128, 20] layout; host reassembles [T, B, 5].
"""

import os
import sys

sys.path.insert(0, "/opt/trn_rl_repo")

from contextlib import ExitStack

import numpy as np

from concourse import bacc, mybir, tile
from concourse.bass_utils import run_bass_kernel_spmd

BETA = 0.9
T, F, H, O = 500, 40, 128, 5
NCORES = 8
BC = 512  # batch per core
NG = BC // H  # 4 batch groups in the L2 batch-major layout
W2COLS = NG * O  # 20
CH = 20  # time steps per DMA chunk (must divide T)
KP = 3 * F + 2  # 122: [xhi; ones; xlo; xhi; ones]
F32 = mybir.dt.float32
F16 = mybir.dt.float16

MULT = mybir.AluOpType.mult
ADD = mybir.AluOpType.add
IS_GT = mybir.AluOpType.is_gt


def build(nc, n_steps=T, ch=CH):
    """Emit the per-core program. x layout: [n_chunks, KP*ch*BC] fp16."""
    n_chunks = n_steps // ch

    x_d = nc.dram_tensor(
        "x_aug", [n_chunks, KP * ch * BC], F16, kind="ExternalInput"
    ).ap()
    w1p_d = nc.dram_tensor("w1p", [KP, H], F16, kind="ExternalInput").ap()
    w2hi_d = nc.dram_tensor("w2hi", [H, O], F16, kind="ExternalInput").ap()
    w2lo_d = nc.dram_tensor("w2lo", [H, O], F16, kind="ExternalInput").ap()
    ones2_d = nc.dram_tensor("ones2", [2, H], F16, kind="ExternalInput").ap()
    c2r_d = nc.dram_tensor("c2rows", [2, W2COLS], F16, kind="ExternalInput").ap()
    out_d = nc.dram_tensor(
        "out", [H, n_steps * W2COLS], F16, kind="ExternalOutput"
    ).ap()

    with tile.TileContext(nc) as tc, ExitStack() as ctx:
        const = ctx.enter_context(tc.tile_pool(name="const", bufs=1))
        state = ctx.enter_context(tc.tile_pool(name="state", bufs=1))
        zq = ctx.enter_context(tc.tile_pool(name="zq", bufs=2))
        xin = ctx.enter_context(tc.tile_pool(name="xin", bufs=2))
        outp = ctx.enter_context(tc.tile_pool(name="outp", bufs=2))
        ps1 = ctx.enter_context(tc.tile_pool(name="ps1", bufs=3, space="PSUM"))
        ps2 = ctx.enter_context(tc.tile_pool(name="ps2", bufs=3, space="PSUM"))

        w1p_s = const.tile([KP, H], F16, tag="w1p")
        w2hi_s = const.tile([H, O], F16, tag="w2hi")
        w2lo_s = const.tile([H, O], F16, tag="w2lo")
        ones2_s = const.tile([2, H], F16, tag="ones2")
        c2r_s = const.tile([2, W2COLS], F16, tag="c2r")
        for s, d in [
            (w1p_s, w1p_d),
            (w2hi_s, w2hi_d),
            (w2lo_s, w2lo_d),
            (ones2_s, ones2_d),
            (c2r_s, c2r_d),
        ]:
            nc.sync.dma_start(out=s[:], in_=d[:])

        # Recurrent state, rotation-buffered.
        NZ = 3
        NS = 4  # sg1 history depth (L2 consumes it LAG steps late)
        z1 = [state.tile([H, BC], F32, tag=f"z1_{p}", name=f"z1_{p}") for p in range(NZ)]
        sg = [state.tile([H, BC], F16, tag=f"sg_{p}", name=f"sg_{p}") for p in range(NS)]
        yy = [
            state.tile([H, W2COLS], F32, tag=f"yy_{p}", name=f"yy_{p}")
            for p in range(NZ)
        ]
        spkinit = state.tile([H, W2COLS], F16, tag="spkinit")

        nc.vector.memset(z1[NZ - 1][:], -1.0)  # m1(0)=0 -> z1=-1
        nc.vector.memset(sg[NS - 1][:], -1.0)  # sign(-1)
        nc.vector.memset(yy[NZ - 1][:], -1.0)  # m2(0)=0 -> Y=-1
        nc.vector.memset(spkinit[:], 0.0)

        # x-chunk DMA row split across the 3 DMA-capable queues
        dma_engines = [nc.sync, nc.gpsimd, nc.scalar]
        row_splits = [0, 41, 81, KP]
        rl = ch * BC  # elements per row in a chunk

        def fetch_chunk(chk):
            xt = xin.tile([KP, ch * BC], F16, tag="xt", name=f"xt{chk}")
            for q in range(3):
                r0, r1 = row_splits[q], row_splits[q + 1]
                dma_engines[q].dma_start(
                    out=xt[r0:r1, :],
                    in_=x_d[chk : chk + 1, r0 * rl : r1 * rl],
                )
            return xt

        # Software-pipelined loop. Layer 2 for step u = t - LAG is emitted
        # LAG iterations late so its chain overlaps the L1 recurrence.
        LAG = 2
        xts = [fetch_chunk(0)]
        ot = None
        spk_prev = spkinit[:]
        p1 = ps1.tile([H, BC], F32, tag="p1")
        nc.tensor.matmul(
            p1[:], w1p_s[:], xts[0][:, 0:BC], start=True, stop=True
        )
        for t in range(n_steps + LAG):
            chk, st = divmod(t, ch)
            if t < n_steps:
                if st == 0 and chk + 1 < n_chunks:
                    xts.append(fetch_chunk(chk + 1))

                # ---- layer 1: update, reset, spike ----
                z1q = zq.tile([H, BC], F32, tag="z1q")
                nc.vector.scalar_tensor_tensor(
                    z1q[:], z1[(t - 1) % NZ][:], BETA, p1[:], MULT, ADD
                )
                nc.vector.scalar_tensor_tensor(
                    z1[t % NZ][:], sg[(t - 1) % NS][:], -0.5, z1q[:], MULT, ADD
                )
                nc.scalar.sign(sg[t % NS][:], z1[t % NZ][:])

                # ---- open step t+1's psum (x-only, independent) ----
                if t + 1 < n_steps:
                    nchk, nst = divmod(t + 1, ch)
                    p1 = ps1.tile([H, BC], F32, tag="p1")
                    nc.tensor.matmul(
                        p1[:],
                        w1p_s[:],
                        xts[nchk][:, nst * BC : (nst + 1) * BC],
                        start=True,
                        stop=True,
                    )

            # ---- layer 2 for step u = t - LAG (batch-major [128, 20]) ----
            u = t - LAG
            if u < 0:
                continue
            uchk, ust = divmod(u, ch)
            if ust == 0:
                ot = outp.tile([H, ch * W2COLS], F16, tag="ot")
            p2t = ps2.tile([H, W2COLS], F32, tag="p2t")
            nc.tensor.matmul(p2t[:], ones2_s[:], c2r_s[:], start=True, stop=False)
            sgu = sg[u % NS]
            for g in range(NG):
                blk = sgu[:, g * H : (g + 1) * H]
                cols = p2t[:, g * O : (g + 1) * O]
                nc.tensor.matmul(cols, blk, w2hi_s[:], start=False, stop=False)
                nc.tensor.matmul(
                    cols, blk, w2lo_s[:], start=False, stop=(g == NG - 1)
                )
            qy = zq.tile([H, W2COLS], F32, tag="qy")
            nc.vector.scalar_tensor_tensor(
                qy[:], yy[(u - 1) % NZ][:], BETA, p2t[:], MULT, ADD
            )
            nc.gpsimd.tensor_tensor(
                yy[u % NZ][:], qy[:], spk_prev, mybir.AluOpType.subtract
            )
            o_slice = ot[:, ust * W2COLS : (ust + 1) * W2COLS]
            nc.vector.tensor_single_scalar(o_slice, yy[u % NZ][:], 0.0, IS_GT)
            spk_prev = o_slice

            if ust == ch - 1:
                nc.sync.dma_start(
                    out=out_d[:, uchk * ch * W2COLS : (uchk + 1) * ch * W2COLS],
                    in_=ot[:],
                )


def _split16(a):
    hi = a.astype(np.float16)
    lo = (a.astype(np.float32) - hi.astype(np.float32)).astype(np.float16)
    return hi, lo


def host_inputs(x, W1, b1, W2, b2, n_steps=T, ch=CH):
    """Shard + precompute all per-core device input arrays."""
    n_chunks = n_steps // ch
    x = np.asarray(x, np.float32)[:, :n_steps, :]
    W1 = np.asarray(W1, np.float32)
    b1 = np.asarray(b1, np.float32)
    W2 = np.asarray(W2, np.float32)
    b2 = np.asarray(b2, np.float32)

    # x: [B, T', F] -> per core [T', F, 512], hi/lo split, packed K=122
    xs = x.reshape(NCORES, BC, n_steps, F).transpose(0, 2, 3, 1)  # [8,T',40,512]
    xhi, xlo = _split16(xs)
    aug = np.empty((NCORES, n_steps, KP, BC), np.float16)
    aug[:, :, :F, :] = xhi
    aug[:, :, F, :] = 1.0
    aug[:, :, F + 1 : 2 * F + 1, :] = xlo
    aug[:, :, 2 * F + 1 : 3 * F + 1, :] = xhi
    aug[:, :, 3 * F + 1, :] = 1.0
    aug = aug.reshape(NCORES, n_chunks, ch, KP, BC).transpose(0, 1, 3, 2, 4)
    aug = np.ascontiguousarray(aug).reshape(NCORES, n_chunks, KP * ch * BC)

    w1hi, w1lo = _split16(W1.T)  # [40, 128]
    bias1 = b1 + BETA - 1.5
    bhi, blo = _split16(bias1)
    # pairing: rows 0..39 xhi*W1hi, 40 ones*bias_hi, 41..80 xlo*W1hi,
    #          81..120 xhi*W1lo, 121 ones*bias_lo
    w1p = np.concatenate(
        [w1hi, bhi[None, :], w1hi, w1lo, blo[None, :]], axis=0
    )  # [122, 128]

    w2hi, w2lo = _split16(0.5 * W2.T)  # [128, 5]

    c2 = 0.5 * W2.sum(axis=1) + b2 + BETA - 1.0  # per-o constant, added each step
    c2hi, c2lo = _split16(c2)
    c2rows = np.stack([np.tile(c2hi, NG), np.tile(c2lo, NG)])  # [2, 20]
    ones2 = np.ones((2, H), np.float16)

    shared = {
        "w1p": np.ascontiguousarray(w1p),
        "w2hi": np.ascontiguousarray(w2hi),
        "w2lo": np.ascontiguousarray(w2lo),
        "ones2": ones2,
        "c2rows": np.ascontiguousarray(c2rows),
    }
    return [{"x_aug": aug[c], **shared} for c in range(NCORES)]


def assemble(results, n_steps=T):
    """[128, T'*20] fp16 0/1 batch-major per core -> [T', B, O] float32."""
    outs = []
    for r in results:
        a = np.asarray(r["out"]).reshape(H, n_steps, NG, O)  # [p, t, g, o]
        # b = g*128 + p
        outs.append((a > 0).transpose(1, 2, 0, 3).reshape(n_steps, BC, O))
    return np.concatenate(outs, axis=1).astype(np.float32)


LAST_RESULT = None  # BassKernelResults of the most recent run (for profiling)


def kernel(x, W1, b1, W2, b2):
    global LAST_RESULT
    in_maps = host_inputs(x, W1, b1, W2, b2)
    nc = bacc.Bacc("TRN2", target_bir_lowering=False, debug=False)
    build(nc)
    nc.compile()
    LAST_RESULT = run_bass_kernel_spmd(nc, in_maps, list(range(NCORES)))
    return assemble(LAST_RESULT.results)


# revision 11
# speedup vs baseline: 2.0714x; 2.0714x over previous
"""AudioSNN Trainium2 kernel (v6: packed fp16-split matmuls, batch-major L2).

Two-layer leaky-integrate-and-fire SNN (snntorch Leaky, reset-by-subtract),
T=500 recurrent steps over batch 4096, data-parallel over 8 NeuronCores
(512 batch elements per core).

Math (per step t, reference):
    cur1 = x_t @ W1.T + b1
    m1   = beta*m1 + cur1 - spk1[t-1]
    spk1 = H(m1 - 1)
    cur2 = spk1 @ W2.T + b2
    m2   = beta*m2 + cur2 - spk2[t-1]
    spk2 = H(m2 - 1)    -> output [T, B, 5]

Device formulation (per core, fp16 matmuls with hi/lo splits -> ~1e-7
error, f32 state):

  L1 (feature-major [128, 512], spikes as sgn = sign(z1) in {-1,+1}):
    p1 = one packed matmul (K=122):
         W1hi^T@xhi + bias_hi + W1hi^T@xlo + W1lo^T@xhi + bias_lo
    z1q  = beta*z1[t-1] + p1          (DVE stt)
    z1[t] = -0.5*sg1[t-1] + z1q       (DVE stt: reset-by-subtract)
    sg1[t] = Sign(z1[t]) fp16         (ACT)

  L2 (batch-major [128, 20]: partition = b%128, free = 5*(b//128)+o,
      spikes as 0/1):
    p2t = (ones2^T@c2rows)            (K=2 matmul: per-o constant c2)
        + per batch group g: sg1[:,g]^T @ w2hi + sg1[:,g]^T @ w2lo
          (spikes loaded as PE weights; out partitions = batch)
    q    = beta*Y[u-1] + p2t          (DVE stt)
    Y[u] = q - spk[u-1]               (GpSimd tensor_tensor, fp16 0/1 spk)
    spk[u] = (Y[u] > 0) fp16          (DVE tensor_single_scalar) -> output

  L2 runs LAG=2 steps behind L1 so its chain overlaps the L1 recurrence.
  Output is 0/1 fp16 in [128, 20] layout; host reassembles [T, B, 5].
"""

import os
import sys

sys.path.insert(0, "/opt/trn_rl_repo")

from contextlib import ExitStack

import numpy as np

from concourse import bacc, mybir, tile
from concourse.bass_utils import run_bass_kernel_spmd

BETA = 0.9
T, F, H, O = 500, 40, 128, 5
NCORES = 8
BC = 512  # batch per core
NG = BC // H  # 4 batch groups in the L2 batch-major layout
W2COLS = NG * O  # 20
CH = 25  # time steps per DMA chunk (must divide T)
KA = F + 1  # 41: [xhi; ones]
KP = 2 * F + 1  # 81: [xhi; ones; xlo]
F32 = mybir.dt.float32
F16 = mybir.dt.float16

MULT = mybir.AluOpType.mult
ADD = mybir.AluOpType.add
IS_GT = mybir.AluOpType.is_gt


def build(nc, n_steps=T, ch=CH):
    """Emit the per-core program. x layout: [n_chunks, KP*ch*BC] fp16."""
    n_chunks = n_steps // ch

    x_d = nc.dram_tensor(
        "x_aug", [n_chunks, KP * ch * BC], F16, kind="ExternalInput"
    ).ap()
    w1a_d = nc.dram_tensor("w1a", [KA, H], F16, kind="ExternalInput").ap()
    w1c_d = nc.dram_tensor("w1c", [KP, H], F16, kind="ExternalInput").ap()
    w2hi_d = nc.dram_tensor("w2hi", [H, O], F16, kind="ExternalInput").ap()
    w2lo_d = nc.dram_tensor("w2lo", [H, O], F16, kind="ExternalInput").ap()
    c2t_d = nc.dram_tensor("c2t", [H, W2COLS], F32, kind="ExternalInput").ap()
    out_d = nc.dram_tensor(
        "out", [H, n_steps * W2COLS], F16, kind="ExternalOutput"
    ).ap()

    with tile.TileContext(nc) as tc, ExitStack() as ctx:
        const = ctx.enter_context(tc.tile_pool(name="const", bufs=1))
        state = ctx.enter_context(tc.tile_pool(name="state", bufs=1))
        zq = ctx.enter_context(tc.tile_pool(name="zq", bufs=2))
        xin = ctx.enter_context(tc.tile_pool(name="xin", bufs=3))
        outp = ctx.enter_context(tc.tile_pool(name="outp", bufs=2))
        ps1 = ctx.enter_context(tc.tile_pool(name="ps1", bufs=3, space="PSUM"))
        ps2 = ctx.enter_context(tc.tile_pool(name="ps2", bufs=3, space="PSUM"))

        w1a_s = const.tile([KA, H], F16, tag="w1a")
        w1c_s = const.tile([KP, H], F16, tag="w1c")
        w2hi_s = const.tile([H, O], F16, tag="w2hi")
        w2lo_s = const.tile([H, O], F16, tag="w2lo")
        c2t_s = const.tile([H, W2COLS], F32, tag="c2t")
        for s, d in [
            (w1a_s, w1a_d),
            (w1c_s, w1c_d),
            (w2hi_s, w2hi_d),
            (w2lo_s, w2lo_d),
            (c2t_s, c2t_d),
        ]:
            nc.sync.dma_start(out=s[:], in_=d[:])

        # Recurrent state, rotation-buffered.
        NZ = 3
        NS = 4  # sg1 history depth (L2 consumes it LAG steps late)
        z1 = [state.tile([H, BC], F32, tag=f"z1_{p}", name=f"z1_{p}") for p in range(NZ)]
        sg = [state.tile([H, BC], F16, tag=f"sg_{p}", name=f"sg_{p}") for p in range(NS)]
        yy = [
            state.tile([H, W2COLS], F32, tag=f"yy_{p}", name=f"yy_{p}")
            for p in range(NZ)
        ]
        spkinit = state.tile([H, W2COLS], F16, tag="spkinit")

        nc.vector.memset(z1[NZ - 1][:], -1.0)  # m1(0)=0 -> z1=-1
        nc.vector.memset(sg[NS - 1][:], -1.0)  # sign(-1)
        nc.vector.memset(yy[NZ - 1][:], -1.0)  # m2(0)=0 -> Y=-1
        nc.vector.memset(spkinit[:], 0.0)

        # x-chunk DMA row split across the 3 DMA-capable queues
        dma_engines = [nc.sync, nc.gpsimd, nc.scalar]
        row_splits = [0, 27, 54, KP]
        rl = ch * BC  # elements per row in a chunk

        def fetch_chunk(chk):
            xt = xin.tile([KP, ch * BC], F16, tag="xt", name=f"xt{chk}")
            for q in range(3):
                r0, r1 = row_splits[q], row_splits[q + 1]
                dma_engines[q].dma_start(
                    out=xt[r0:r1, :],
                    in_=x_d[chk : chk + 1, r0 * rl : r1 * rl],
                )
            return xt

        # Software-pipelined loop. Layer 2 for step u = t - LAG is emitted
        # LAG iterations late so its chain overlaps the L1 recurrence.
        LAG = 2
        xts = [fetch_chunk(0)]
        if n_chunks > 1:
            xts.append(fetch_chunk(1))
        ot = None
        spk_prev = spkinit[:]

        def mm1(p1, xt, st):
            sl = slice(st * BC, (st + 1) * BC)
            nc.tensor.matmul(p1[:], w1a_s[:], xt[:KA, sl], start=True, stop=False)
            nc.tensor.matmul(p1[:], w1c_s[:], xt[:KP, sl], start=False, stop=True)

        p1 = ps1.tile([H, BC], F32, tag="p1")
        mm1(p1, xts[0], 0)
        for t in range(n_steps + LAG):
            chk, st = divmod(t, ch)
            if t < n_steps:
                if st == 0 and chk + 2 < n_chunks:
                    xts.append(fetch_chunk(chk + 2))

                # ---- layer 1: update, reset, spike ----
                z1q = zq.tile([H, BC], F32, tag="z1q")
                nc.vector.scalar_tensor_tensor(
                    z1q[:], z1[(t - 1) % NZ][:], BETA, p1[:], MULT, ADD
                )
                nc.vector.scalar_tensor_tensor(
                    z1[t % NZ][:], sg[(t - 1) % NS][:], -0.5, z1q[:], MULT, ADD
                )
                nc.scalar.sign(sg[t % NS][:], z1[t % NZ][:])

                # ---- open step t+1's psum (x-only, independent) ----
                if t + 1 < n_steps:
                    nchk, nst = divmod(t + 1, ch)
                    p1 = ps1.tile([H, BC], F32, tag="p1")
                    mm1(p1, xts[nchk], nst)

            # ---- layer 2 for step u = t - LAG (batch-major [128, 20]) ----
            u = t - LAG
            if u < 0:
                continue
            uchk, ust = divmod(u, ch)
            if ust == 0:
                ot = outp.tile([H, ch * W2COLS], F16, tag="ot")
            p2t = ps2.tile([H, W2COLS], F32, tag="p2t")
            sgu = sg[u % NS]
            for g in range(NG):
                blk = sgu[:, g * H : (g + 1) * H]
                cols = p2t[:, g * O : (g + 1) * O]
                nc.tensor.matmul(cols, blk, w2hi_s[:], start=(g == 0), stop=False)
                nc.tensor.matmul(
                    cols, blk, w2lo_s[:], start=False, stop=(g == NG - 1)
                )
            qy = zq.tile([H, W2COLS], F32, tag="qy")
            nc.vector.scalar_tensor_tensor(
                qy[:], yy[(u - 1) % NZ][:], BETA, p2t[:], MULT, ADD
            )
            qc = zq.tile([H, W2COLS], F32, tag="qc")
            nc.gpsimd.tensor_tensor(qc[:], qy[:], c2t_s[:], ADD)
            nc.gpsimd.tensor_tensor(
                yy[u % NZ][:], qc[:], spk_prev, mybir.AluOpType.subtract
            )
            o_slice = ot[:, ust * W2COLS : (ust + 1) * W2COLS]
            nc.vector.tensor_single_scalar(o_slice, yy[u % NZ][:], 0.0, IS_GT)
            spk_prev = o_slice

            if ust == ch - 1:
                nc.sync.dma_start(
                    out=out_d[:, uchk * ch * W2COLS : (uchk + 1) * ch * W2COLS],
                    in_=ot[:],
                )


def _split16(a):
    hi = a.astype(np.float16)
    lo = (a.astype(np.float32) - hi.astype(np.float32)).astype(np.float16)
    return hi, lo


def host_inputs(x, W1, b1, W2, b2, n_steps=T, ch=CH):
    """Shard + precompute all per-core device input arrays."""
    n_chunks = n_steps // ch
    x = np.asarray(x, np.float32)[:, :n_steps, :]
    W1 = np.asarray(W1, np.float32)
    b1 = np.asarray(b1, np.float32)
    W2 = np.asarray(W2, np.float32)
    b2 = np.asarray(b2, np.float32)

    # x: [B, T', F] -> per core [T', F, 512], hi/lo split, K=81 rows
    xs = x.reshape(NCORES, BC, n_steps, F).transpose(0, 2, 3, 1)  # [8,T',40,512]
    xhi, xlo = _split16(xs)
    aug = np.empty((NCORES, n_steps, KP, BC), np.float16)
    aug[:, :, :F, :] = xhi
    aug[:, :, F, :] = 1.0
    aug[:, :, F + 1 :, :] = xlo
    aug = aug.reshape(NCORES, n_chunks, ch, KP, BC).transpose(0, 1, 3, 2, 4)
    aug = np.ascontiguousarray(aug).reshape(NCORES, n_chunks, KP * ch * BC)

    w1hi, w1lo = _split16(W1.T)  # [40, 128]
    bias1 = b1 + BETA - 1.5
    bhi, blo = _split16(bias1)
    # mm1a (K=41): xhi*W1hi + ones*bias_hi
    # mm1c (K=81): xhi*W1lo + ones*bias_lo + xlo*W1hi
    w1a = np.concatenate([w1hi, bhi[None, :]], axis=0)  # [41, 128]
    w1c = np.concatenate([w1lo, blo[None, :], w1hi], axis=0)  # [81, 128]

    w2hi, w2lo = _split16(0.5 * W2.T)  # [128, 5]

    c2 = 0.5 * W2.sum(axis=1) + b2 + BETA - 1.0  # per-o constant, added each step
    c2t = np.tile(np.tile(c2, NG)[None, :], (H, 1)).astype(np.float32)  # [128, 20]

    shared = {
        "w1a": np.ascontiguousarray(w1a),
        "w1c": np.ascontiguousarray(w1c),
        "w2hi": np.ascontiguousarray(w2hi),
        "w2lo": np.ascontiguousarray(w2lo),
        "c2t": c2t,
    }
    return [{"x_aug": aug[c], **shared} for c in range(NCORES)]


def assemble(results, n_steps=T):
    """[128, T'*20] fp16 0/1 batch-major per core -> [T', B, O] float32."""
    outs = []
    for r in results:
        a = np.asarray(r["out"]).reshape(H, n_steps, NG, O)  # [p, t, g, o]
        # b = g*128 + p
        outs.append((a > 0).transpose(1, 2, 0, 3).reshape(n_steps, BC, O))
    return np.concatenate(outs, axis=1).astype(np.float32)


LAST_RESULT = None  # BassKernelResults of the most recent run (for profiling)


def kernel(x, W1, b1, W2, b2):
    global LAST_RESULT
    in_maps = host_inputs(x, W1, b1, W2, b2)
    nc = bacc.Bacc("TRN2", target_bir_lowering=False, debug=False)
    build(nc)
    nc.compile()
    LAST_RESULT = run_bass_kernel_spmd(nc, in_maps, list(range(NCORES)))
    return assemble(LAST_RESULT.results)


# revision 12
# speedup vs baseline: 2.0776x; 1.0030x over previous
"""AudioSNN Trainium2 kernel (v6: packed fp16-split matmuls, batch-major L2).

Two-layer leaky-integrate-and-fire SNN (snntorch Leaky, reset-by-subtract),
T=500 recurrent steps over batch 4096, data-parallel over 8 NeuronCores
(512 batch elements per core).

Math (per step t, reference):
    cur1 = x_t @ W1.T + b1
    m1   = beta*m1 + cur1 - spk1[t-1]
    spk1 = H(m1 - 1)
    cur2 = spk1 @ W2.T + b2
    m2   = beta*m2 + cur2 - spk2[t-1]
    spk2 = H(m2 - 1)    -> output [T, B, 5]

Device formulation (per core, fp16 matmuls with hi/lo splits -> ~1e-7
error, f32 state):

  L1 (feature-major [128, 512], spikes as sgn = sign(z1) in {-1,+1}):
    p1 = one packed matmul (K=122):
         W1hi^T@xhi + bias_hi + W1hi^T@xlo + W1lo^T@xhi + bias_lo
    z1q  = beta*z1[t-1] + p1          (DVE stt)
    z1[t] = -0.5*sg1[t-1] + z1q       (DVE stt: reset-by-subtract)
    sg1[t] = Sign(z1[t]) fp16         (ACT)

  L2 (batch-major [128, 20]: partition = b%128, free = 5*(b//128)+o,
      spikes as 0/1):
    p2t = (ones2^T@c2rows)            (K=2 matmul: per-o constant c2)
        + per batch group g: sg1[:,g]^T @ w2hi + sg1[:,g]^T @ w2lo
          (spikes loaded as PE weights; out partitions = batch)
    q    = beta*Y[u-1] + p2t          (DVE stt)
    Y[u] = q - spk[u-1]               (GpSimd tensor_tensor, fp16 0/1 spk)
    spk[u] = (Y[u] > 0) fp16          (DVE tensor_single_scalar) -> output

  L2 runs LAG=2 steps behind L1 so its chain overlaps the L1 recurrence.
  Output is 0/1 fp16 in [128, 20] layout; host reassembles [T, B, 5].
"""

import os
import sys

sys.path.insert(0, "/opt/trn_rl_repo")

from contextlib import ExitStack

import numpy as np

from concourse import bacc, mybir, tile
from concourse.bass_utils import run_bass_kernel_spmd

BETA = 0.9
T, F, H, O = 500, 40, 128, 5
NCORES = 8
BC = 512  # batch per core
NG = BC // H  # 4 batch groups in the L2 batch-major layout
W2COLS = NG * O  # 20
CH = 25  # time steps per DMA chunk (must divide T)
KA = F + 1  # 41: [xhi; ones]
KP = 2 * F + 1  # 81: [xhi; ones; xlo]
F32 = mybir.dt.float32
F16 = mybir.dt.float16

MULT = mybir.AluOpType.mult
ADD = mybir.AluOpType.add
IS_GT = mybir.AluOpType.is_gt


def build(nc, n_steps=T, ch=CH):
    """Emit the per-core program. x layout: [n_chunks, KP*ch*BC] fp16."""
    n_chunks = n_steps // ch

    x_d = nc.dram_tensor(
        "x_aug", [n_chunks, KP * ch * BC], F16, kind="ExternalInput"
    ).ap()
    w1a_d = nc.dram_tensor("w1a", [KA, H], F16, kind="ExternalInput").ap()
    w1c_d = nc.dram_tensor("w1c", [KP, H], F16, kind="ExternalInput").ap()
    w2hi_d = nc.dram_tensor("w2hi", [H, O], F16, kind="ExternalInput").ap()
    w2lo_d = nc.dram_tensor("w2lo", [H, O], F16, kind="ExternalInput").ap()
    c2t_d = nc.dram_tensor("c2t", [H, W2COLS], F32, kind="ExternalInput").ap()
    out_d = nc.dram_tensor(
        "out", [H, n_steps * W2COLS], F16, kind="ExternalOutput"
    ).ap()

    with tile.TileContext(nc) as tc, ExitStack() as ctx:
        const = ctx.enter_context(tc.tile_pool(name="const", bufs=1))
        state = ctx.enter_context(tc.tile_pool(name="state", bufs=1))
        zq = ctx.enter_context(tc.tile_pool(name="zq", bufs=2))
        xin = ctx.enter_context(tc.tile_pool(name="xin", bufs=3))
        outp = ctx.enter_context(tc.tile_pool(name="outp", bufs=2))
        ps1 = ctx.enter_context(tc.tile_pool(name="ps1", bufs=3, space="PSUM"))
        ps2 = ctx.enter_context(tc.tile_pool(name="ps2", bufs=3, space="PSUM"))

        w1a_s = const.tile([KA, H], F16, tag="w1a")
        w1c_s = const.tile([KP, H], F16, tag="w1c")
        w2hi_s = const.tile([H, O], F16, tag="w2hi")
        w2lo_s = const.tile([H, O], F16, tag="w2lo")
        c2t_s = const.tile([H, W2COLS], F32, tag="c2t")
        for s, d in [
            (w1a_s, w1a_d),
            (w1c_s, w1c_d),
            (w2hi_s, w2hi_d),
            (w2lo_s, w2lo_d),
            (c2t_s, c2t_d),
        ]:
            nc.sync.dma_start(out=s[:], in_=d[:])

        # Recurrent state, rotation-buffered.
        NZ = 3
        NS = 4  # sg1 history depth (L2 consumes it LAG steps late)
        z1 = [state.tile([H, BC], F32, tag=f"z1_{p}", name=f"z1_{p}") for p in range(NZ)]
        sg = [state.tile([H, BC], F16, tag=f"sg_{p}", name=f"sg_{p}") for p in range(NS)]
        yy = [
            state.tile([H, W2COLS], F32, tag=f"yy_{p}", name=f"yy_{p}")
            for p in range(NZ)
        ]
        spkinit = state.tile([H, W2COLS], F16, tag="spkinit")

        nc.vector.memset(z1[NZ - 1][:], -1.0)  # m1(0)=0 -> z1=-1
        nc.vector.memset(sg[NS - 1][:], -1.0)  # sign(-1)
        nc.vector.memset(yy[NZ - 1][:], -1.0)  # m2(0)=0 -> Y=-1
        nc.vector.memset(spkinit[:], 0.0)

        # x-chunk DMA row split across the 3 DMA-capable queues
        dma_engines = [nc.sync, nc.gpsimd, nc.scalar]
        row_splits = [0, 27, 54, KP]
        rl = ch * BC  # elements per row in a chunk

        def fetch_chunk(chk):
            xt = xin.tile([KP, ch * BC], F16, tag="xt", name=f"xt{chk}")
            for q in range(3):
                r0, r1 = row_splits[q], row_splits[q + 1]
                dma_engines[q].dma_start(
                    out=xt[r0:r1, :],
                    in_=x_d[chk : chk + 1, r0 * rl : r1 * rl],
                )
            return xt

        # Software-pipelined loop. Layer 2 for step u = t - LAG is emitted
        # LAG iterations late so its chain overlaps the L1 recurrence.
        LAG = 2
        xts = [fetch_chunk(0)]
        if n_chunks > 1:
            xts.append(fetch_chunk(1))
        ot = None
        spk_prev = spkinit[:]

        def mm1(p1, xt, st):
            sl = slice(st * BC, (st + 1) * BC)
            nc.tensor.matmul(p1[:], w1a_s[:], xt[:KA, sl], start=True, stop=False)
            nc.tensor.matmul(p1[:], w1c_s[:], xt[:KP, sl], start=False, stop=True)

        p1 = ps1.tile([H, BC], F32, tag="p1")
        mm1(p1, xts[0], 0)
        for t in range(n_steps + LAG):
            chk, st = divmod(t, ch)
            if t < n_steps:
                if st == 0 and chk + 2 < n_chunks:
                    xts.append(fetch_chunk(chk + 2))

                # ---- layer 1: update, reset, spike ----
                z1q = zq.tile([H, BC], F32, tag="z1q")
                nc.vector.scalar_tensor_tensor(
                    z1q[:], z1[(t - 1) % NZ][:], BETA, p1[:], MULT, ADD
                )
                nc.vector.scalar_tensor_tensor(
                    z1[t % NZ][:], sg[(t - 1) % NS][:], -0.5, z1q[:], MULT, ADD
                )
                nc.scalar.sign(sg[t % NS][:], z1[t % NZ][:])

                # ---- open step t+1's psum (x-only, independent) ----
                if t + 1 < n_steps:
                    nchk, nst = divmod(t + 1, ch)
                    p1 = ps1.tile([H, BC], F32, tag="p1")
                    mm1(p1, xts[nchk], nst)

            # ---- layer 2 for step u = t - LAG (batch-major [128, 20]) ----
            u = t - LAG
            if u < 0:
                continue
            uchk, ust = divmod(u, ch)
            if ust == 0:
                ot = outp.tile([H, ch * W2COLS], F16, tag="ot")
            p2t = ps2.tile([H, W2COLS], F32, tag="p2t")
            sgu = sg[u % NS]
            for g in range(NG):
                blk = sgu[:, g * H : (g + 1) * H]
                cols = p2t[:, g * O : (g + 1) * O]
                nc.tensor.matmul(cols, blk, w2hi_s[:], start=(g == 0), stop=False)
                nc.tensor.matmul(
                    cols, blk, w2lo_s[:], start=False, stop=(g == NG - 1)
                )
            qy = zq.tile([H, W2COLS], F32, tag="qy")
            nc.vector.scalar_tensor_tensor(
                qy[:], yy[(u - 1) % NZ][:], BETA, p2t[:], MULT, ADD
            )
            qc = zq.tile([H, W2COLS], F32, tag="qc")
            nc.gpsimd.tensor_tensor(qc[:], qy[:], c2t_s[:], ADD)
            nc.gpsimd.tensor_tensor(
                yy[u % NZ][:], qc[:], spk_prev, mybir.AluOpType.subtract
            )
            o_slice = ot[:, ust * W2COLS : (ust + 1) * W2COLS]
            nc.gpsimd.tensor_single_scalar(o_slice, yy[u % NZ][:], 0.0, IS_GT)
            spk_prev = o_slice

            if ust == ch - 1:
                nc.sync.dma_start(
                    out=out_d[:, uchk * ch * W2COLS : (uchk + 1) * ch * W2COLS],
                    in_=ot[:],
                )


def _split16(a):
    hi = a.astype(np.float16)
    lo = (a.astype(np.float32) - hi.astype(np.float32)).astype(np.float16)
    return hi, lo


def host_inputs(x, W1, b1, W2, b2, n_steps=T, ch=CH):
    """Shard + precompute all per-core device input arrays."""
    n_chunks = n_steps // ch
    x = np.asarray(x, np.float32)[:, :n_steps, :]
    W1 = np.asarray(W1, np.float32)
    b1 = np.asarray(b1, np.float32)
    W2 = np.asarray(W2, np.float32)
    b2 = np.asarray(b2, np.float32)

    # x: [B, T', F] -> per core [T', F, 512], hi/lo split, K=81 rows
    xs = x.reshape(NCORES, BC, n_steps, F).transpose(0, 2, 3, 1)  # [8,T',40,512]
    xhi, xlo = _split16(xs)
    aug = np.empty((NCORES, n_steps, KP, BC), np.float16)
    aug[:, :, :F, :] = xhi
    aug[:, :, F, :] = 1.0
    aug[:, :, F + 1 :, :] = xlo
    aug = aug.reshape(NCORES, n_chunks, ch, KP, BC).transpose(0, 1, 3, 2, 4)
    aug = np.ascontiguousarray(aug).reshape(NCORES, n_chunks, KP * ch * BC)

    w1hi, w1lo = _split16(W1.T)  # [40, 128]
    bias1 = b1 + BETA - 1.5
    bhi, blo = _split16(bias1)
    # mm1a (K=41): xhi*W1hi + ones*bias_hi
    # mm1c (K=81): xhi*W1lo + ones*bias_lo + xlo*W1hi
    w1a = np.concatenate([w1hi, bhi[None, :]], axis=0)  # [41, 128]
    w1c = np.concatenate([w1lo, blo[None, :], w1hi], axis=0)  # [81, 128]

    w2hi, w2lo = _split16(0.5 * W2.T)  # [128, 5]

    c2 = 0.5 * W2.sum(axis=1) + b2 + BETA - 1.0  # per-o constant, added each step
    c2t = np.tile(np.tile(c2, NG)[None, :], (H, 1)).astype(np.float32)  # [128, 20]

    shared = {
        "w1a": np.ascontiguousarray(w1a),
        "w1c": np.ascontiguousarray(w1c),
        "w2hi": np.ascontiguousarray(w2hi),
        "w2lo": np.ascontiguousarray(w2lo),
        "c2t": c2t,
    }
    return [{"x_aug": aug[c], **shared} for c in range(NCORES)]


def assemble(results, n_steps=T):
    """[128, T'*20] fp16 0/1 batch-major per core -> [T', B, O] float32."""
    outs = []
    for r in results:
        a = np.asarray(r["out"]).reshape(H, n_steps, NG, O)  # [p, t, g, o]
        # b = g*128 + p
        outs.append((a > 0).transpose(1, 2, 0, 3).reshape(n_steps, BC, O))
    return np.concatenate(outs, axis=1).astype(np.float32)


LAST_RESULT = None  # BassKernelResults of the most recent run (for profiling)


def kernel(x, W1, b1, W2, b2):
    global LAST_RESULT
    in_maps = host_inputs(x, W1, b1, W2, b2)
    nc = bacc.Bacc("TRN2", target_bir_lowering=False, debug=False)
    build(nc)
    nc.compile()
    LAST_RESULT = run_bass_kernel_spmd(nc, in_maps, list(range(NCORES)))
    return assemble(LAST_RESULT.results)
